# revision 1
# baseline (speedup 1.0000x reference)
"""Trainium2 Bass kernel for the CubeSimulator problem (v2).

Reference: rotate (96,96,96) grids, per-voxel line-of-sight velocity u and
intensity I, Gaussian-KDE cube over 64 velocity bins, then trilinear
downsample (96,96,64) -> (32,64,64).

Exact structure reused from v1 (validated):
 - axis0 downsample (96->32) is a pure selection of rows 3k+1;
 - axis2 downsample (64->64) is the identity;
 - axis1 downsample (96->64) is a 2-tap stencil (0.75/0.25) matmul;
 - exp(L - (v-u)^2/sig^2) = exp(A) * exp(v*B + c_v) with
   A = L + ln(norm) - u^2/sig^2, B = 2u/sig^2, c_v = -v^2/sig^2.

New in v2 (tolerance-aware, rel err ~1.5e-3 vs 2e-2 budget):
 - Coarse-bin KDE: the cube is computed at NC=32 velocity centers and all
   64 reference bins are reconstructed with a ridge-regularized
   least-squares matrix R (a Gaussian with sigma=30 sampled at dv=19 is
   ~3x oversampled; aliasing ~2e-4). Halves the dominant per-bin work.
 - Wrap layout [128, 288]: per-core voxels flat=(px*96+z) are laid out
   partition=flat%128, free=flat//128, using all 128 lanes (elementwise
   engine cost scales with free size only). The z-sum for pixel p covers
   flat [96p, 96p+96), reduced on the (otherwise idle) TensorE with three
   accumulating matmuls per bin whose [128,4] selector stationaries are
   independent of the column triplet (128*3 = 96*4).
 - KDE tiles in bf16: DVE runs 2-byte tensor_tensor at 2x; exp args stay
   fp32 (ACT reads Bt fp32; scale/bias are per-bin immediates/APs).
 - Per-bin path: one ScalarE Exp (scale=vc, bias=c_v AP) + one V/P mult
   by P0 = exp(A). ScalarE is the bottleneck engine; all of prep's
   square/abs/ln/exp stay inside the natural_log_exp_and_others table.

Sharding: 32 needed i-rows split 4-per-core across 8 cores; only the
final (64, 4*64) tile is gathered per core.
"""

import math

import numpy as np

import concourse.bacc as bacc
import concourse.bass as bass
import concourse.mybir as mybir
import concourse.tile as tile
from concourse.bass_utils import run_bass_kernel_spmd

try:
    import ml_dtypes
    _BF16 = np.dtype(ml_dtypes.bfloat16)
except Exception:  # pragma: no cover
    _BF16 = None

G = 96            # up_gal grid size
NV = 64           # reference velocity bins
NC = 28           # coarse KDE bins (reconstructed to NV by matmul)
N_CORES = 8
OUT_I = 32        # selected i rows (axis-0 downsample = row selection)
ROWS_PER_CORE = OUT_I // N_CORES   # 4
PX = ROWS_PER_CORE * G             # 384 pixels per core
NZ = G                             # z depth
NFLAT = PX * NZ                    # 36864 voxels per core
NP128 = 128
NF = NFLAT // NP128                # 288 free columns
NT = NF // 3                       # 96 column triplets (4 pixels each)
OUT_J = 64
RIDGE_LAM = 1e-4

F32 = mybir.dt.float32
BF16 = mybir.dt.bfloat16
AF = mybir.ActivationFunctionType
OP = mybir.AluOpType

LAST_EXEC_NS = None

# tuning knobs
KDE_POOL_MULTS = 6   # of the NC bf16 P0-mults, how many go to GpSimd
PREP_FUSE_STT = False # use scalar_tensor_tensor fusion in prep where legal


def _build_program(ci, si, cr, sr, sig2, lnnorm, vel, safe_affine=None):
    vel = np.asarray(vel, np.float64).reshape(-1)
    vc = np.linspace(float(vel.min()), float(vel.max()), NC)
    usc = -200.0 * si
    if safe_affine is None:
        umax2 = (200.0 * si) ** 2
        safe_affine = not (umax2 / sig2 <= 80.0)
    # bins whose exp argument includes A directly (no P0 mult after).
    # The last bins are affine so the tail after the final exp is mult-free.
    affine_bins = set(range(NC)) if safe_affine else {NC - 2, NC - 1}

    nc = bacc.Bacc("TRN2")

    xs = nc.dram_tensor("xs", [NP128, NF], F32, kind="ExternalInput")
    ys = nc.dram_tensor("ys", [NP128, NF], F32, kind="ExternalInput")
    zs = nc.dram_tensor("zs", [NP128, NF], F32, kind="ExternalInput")
    bc = nc.dram_tensor("bc", [NP128, NC], F32, kind="ExternalInput")
    sc = nc.dram_tensor("sc", [NP128, 12], BF16, kind="ExternalInput")
    wr = nc.dram_tensor("wr", [NP128, 4 * NV], BF16, kind="ExternalInput")
    sm = nc.dram_tensor("sm", [NT, 16 * OUT_J], BF16, kind="ExternalInput")
    idm = nc.dram_tensor("idm", [NT, NT], BF16, kind="ExternalInput")
    out = nc.dram_tensor("out", [OUT_J, ROWS_PER_CORE * NV], F32,
                         kind="ExternalOutput")

    with tile.TileContext(nc) as tc:
        with (
            tc.tile_pool(name="io", bufs=1) as io,
            tc.tile_pool(name="prep", bufs=1) as prep,
            tc.tile_pool(name="kde", bufs=2) as kde,
            tc.tile_pool(name="psum", bufs=1, space="PSUM") as psum,
        ):
            # Preload the one activation table covering ln/exp/abs: avoids
            # two mid-kernel table swaps (~1.3us each) from the inserter's
            # minimal-set choice.
            from concourse.hw_specs import get_activation_tables
            tabs = get_activation_tables(nc.m.arch)
            want = {AF.Ln, AF.Exp, AF.Abs}
            for idx, (tname, funcs) in enumerate(tabs.items()):
                if want.issubset(funcs):
                    ld = mybir.InstLoadActFuncSet(
                        name=nc.scalar.bass.get_next_instruction_name(),
                        act_func_set_id=idx, ins=[], outs=[])
                    nc.scalar.add_instruction(ld)
                    break

            xt = io.tile([NP128, NF], F32, tag="xt")
            yt = io.tile([NP128, NF], F32, tag="yt")
            zt = io.tile([NP128, NF], F32, tag="zt")
            # input DMAs split in column halves so prep's first-half chain
            # starts ~0.7us earlier (HWDGE is serial; sem prop is ~0.9us)
            HF = NF // 2
            def half(ap, h):
                return ap[:, h * HF:(h + 1) * HF]
            for h in range(2):
                nc.sync.dma_start(out=half(zt, h), in_=half(zs, h))
                nc.sync.dma_start(out=half(xt, h), in_=half(xs, h))
                nc.sync.dma_start(out=half(yt, h), in_=half(ys, h))
            bct = io.tile([NP128, NC], F32, tag="bct")
            nc.sync.dma_start(out=bct[:], in_=bc[:])
            sct = io.tile([NP128, 12], BF16, tag="sct")
            nc.sync.dma_start(out=sct[:], in_=sc[:])
            idt = io.tile([NT, NT], BF16, tag="idt")
            nc.sync.dma_start(out=idt[:], in_=idm[:])
            wrt = io.tile([NP128, 4 * NV], BF16, tag="wrt")
            nc.sync.dma_start(out=wrt[:], in_=wr[:])
            smt = io.tile([NT, 16 * OUT_J], BF16, tag="smt")
            nc.sync.dma_start(out=smt[:], in_=sm[:])

            def vtile(name):
                return prep.tile([NP128, NF], F32, tag=name, name=name)

            # Prep, pipelined over two column halves to halve the serial
            # dependency chain's latency before the first KDE exp.
            # Only rx and rz legs are needed: r^2 = x^2+y^2+z^2 - rz^2
            # (rotation preserves |v|), so the ry leg is dropped.
            rx, rz = vtile("rx"), vtile("rz")
            sqx, sqy, sqz = vtile("sqx"), vtile("sqy"), vtile("sqz")
            s2, s3 = vtile("s2"), vtile("s3")
            xc, yc, t5, zb = vtile("xc"), vtile("yc"), vtile("t5"), vtile("zb")
            rzq, q, qs = vtile("rzq"), vtile("q"), vtile("qs")
            ya, xa = vtile("ya"), vtile("xa")
            lnq, r, er = vtile("lnq"), vtile("r"), vtile("er")
            ed, den, rec = vtile("ed"), vtile("den"), vtile("rec")
            num, t1, u0 = vtile("num"), vtile("t1"), vtile("u0")
            az, rterm, Lt = vtile("az"), vtile("rterm"), vtile("Lt")
            s1, ssq, At = vtile("s1"), vtile("ssq"), vtile("At")
            V, P, S = nc.vector, nc.gpsimd, nc.scalar
            for h in range(2):
                # geometry: q = x^2+y^2+z^2 - rz^2 and rx, per half
                V.tensor_mul(half(sqz, h), half(zt, h), half(zt, h))
                P.tensor_scalar_mul(half(zb, h), half(zt, h), ci)
                P.tensor_scalar_mul(half(xc, h), half(xt, h), si * sr)
                V.tensor_mul(half(sqx, h), half(xt, h), half(xt, h))
                P.tensor_scalar_mul(half(yc, h), half(yt, h), si * cr)
                P.tensor_add(half(t5, h), half(xc, h), half(yc, h))
                V.tensor_mul(half(sqy, h), half(yt, h), half(yt, h))
                V.tensor_add(half(s2, h), half(sqx, h), half(sqy, h))
                V.tensor_add(half(s3, h), half(s2, h), half(sqz, h))
                V.tensor_add(half(rz, h), half(t5, h), half(zb, h))
                V.tensor_mul(half(rzq, h), half(rz, h), half(rz, h))
                V.tensor_sub(half(q, h), half(s3, h), half(rzq, h))
                V.tensor_scalar_max(half(qs, h), half(q, h), 1e-35)
                V.tensor_scalar_mul(half(ya, h), half(yt, h), -sr)
                V.tensor_scalar_mul(half(xa, h), half(xt, h), cr)
                V.tensor_add(half(rx, h), half(xa, h), half(ya, h))
                # ACT ladder for this half
                S.activation(half(az, h), half(rz, h), AF.Abs)
                S.activation(half(lnq, h), half(qs, h), AF.Ln)
                S.activation(half(r, h), half(lnq, h), AF.Exp, scale=0.5)
                S.activation(half(er, h), half(r, h), AF.Exp)
                P.tensor_scalar_add(half(num, h), half(er, h), -1.0)
            for h in range(2):
                # u0 = rx*(e^r-1) / (r*(e^r+1)), per half
                V.tensor_scalar_add(half(ed, h), half(er, h), 1.0)
                V.tensor_mul(half(den, h), half(ed, h), half(r, h))
                V.reciprocal(half(rec, h), half(den, h))
                V.tensor_mul(half(t1, h), half(rx, h), half(num, h))
                V.tensor_mul(half(u0, h), half(t1, h), half(rec, h))
            for h in range(2):
                # A = -r/3 - 2|rz| + lnnorm - (u*usc/sig)^2
                P.tensor_scalar(half(rterm, h), half(r, h), -1.0 / 3.0,
                                lnnorm, OP.mult, OP.add)
                P.tensor_scalar_mul(half(az, h), half(az, h), -2.0)
                P.tensor_add(half(Lt, h), half(az, h), half(rterm, h))
                V.tensor_scalar_mul(half(s1, h), half(u0, h),
                                    usc / math.sqrt(sig2))
                V.tensor_mul(half(ssq, h), half(s1, h), half(s1, h))
                V.tensor_sub(half(At, h), half(Lt, h), half(ssq, h))

            P0t = vtile("P0t")
            P0b = prep.tile([NP128, NF], BF16, tag="P0b", name="P0b")

            # KDE over NC coarse bins; Op[t, 4b+m] accumulates the z-sums
            # (pixel p=4t+m of bin b) via three selector matmuls per bin.
            Op = psum.tile([NT, 4 * NC], F32)
            POOL_MULT_BINS = set(range(0, 2 * KDE_POOL_MULTS, 2))
            esc = usc * 2.0 / sig2
            pending = []

            def emit_tail(b, Ew):
                if b not in affine_bins:
                    e1 = Ew
                    Ew = kde.tile([NP128, NT, 3], BF16, tag="Ew", bufs=6,
                                  name=f"Ew{b}")
                    eng = nc.gpsimd if b in POOL_MULT_BINS else nc.vector
                    eng.tensor_mul(Ew[:], e1[:], P0b[:])
                for c in range(3):
                    nc.tensor.matmul(Op[:, 4 * b:4 * b + 4],
                                     Ew[:, :, c],
                                     sct[:, 4 * c:4 * c + 4],
                                     start=(c == 0), stop=(c == 2))

            # affine args for the designated bins (DVE slack, early emit is
            # fine: they only need u0 and At)
            aargs = {}
            for b in sorted(affine_bins):
                cvb = float(-vc[b] * vc[b] / sig2)
                tmp = kde.tile([NP128, NF], F32, tag=f"tmp{b}", bufs=1,
                               name=f"tmp{b}")
                nc.vector.tensor_scalar(tmp[:], u0[:], float(vc[b]) * esc,
                                        cvb, OP.mult, OP.add)
                arg = kde.tile([NP128, NF], F32, tag=f"arg{b}", bufs=1,
                               name=f"arg{b}")
                nc.vector.tensor_add(arg[:], tmp[:], At[:])
                aargs[b] = arg

            for b in range(NC):
                vv = float(vc[b])
                Ewd = kde.tile([NP128, NT, 3], BF16,
                               tag="Ew" if b in affine_bins else "e1",
                               bufs=6, name=f"e1{b}")
                if b in affine_bins:
                    nc.scalar.activation(Ewd[:], aargs[b][:], AF.Exp)
                else:
                    nc.scalar.activation(Ewd[:], u0[:], AF.Exp,
                                         scale=vv * esc,
                                         bias=bct[:, b:b + 1])
                pending.append((b, Ewd))
                if b == 1 and not safe_affine:
                    # P0 exp lands on ACT only now so the first KDE exps
                    # (gated only on u0) are not head-of-line blocked on At
                    nc.scalar.activation(P0t[:], At[:], AF.Exp)
                    nc.vector.tensor_copy(P0b[:], P0t[:])
                    for item in pending:
                        emit_tail(*item)
                    pending = []
                elif b > 1 or safe_affine:
                    for item in pending:
                        emit_tail(*item)
                    pending = []

                if b == 15:
                    # first-chunk rearrange + W-matmul overlap later bins
                    Ops0 = io.tile([NT, 64], BF16, tag="Ops0")
                    nc.vector.tensor_copy(Ops0[:], Op[:, 0:64])
                    Tp = psum.tile([4 * NC, NT], BF16)
                    nc.tensor.transpose(Tp[0:64, :], Ops0[:], idt[:])
                    Os = io.tile([NP128, NT], BF16, tag="Os")
                    nc.vector.tensor_copy(Os[0:64, :], Tp[0:64, :])
                    out1 = psum.tile([NT, 4 * NV], F32)
                    nc.tensor.matmul(out1[:], Os[0:64, :],
                                     wrt[0:64, :],
                                     start=True, stop=False)
            for item in pending:
                emit_tail(*item)

            # second-half rearrange, then cube2[px, v] closes in out1
            W2 = 4 * NC - 64
            Ops1 = io.tile([NT, W2], BF16, tag="Ops1")
            nc.vector.tensor_copy(Ops1[:], Op[:, 64:4 * NC])
            nc.tensor.transpose(Tp[64:4 * NC, :], Ops1[:], idt[:])
            nc.vector.tensor_copy(Os[64:4 * NC, :], Tp[64:4 * NC, :])
            nc.tensor.matmul(out1[:], Os[64:4 * NC, :],
                             wrt[64:4 * NC, :],
                             start=False, stop=True)

            Os1 = io.tile([NT, 4 * NV], BF16, tag="Os1")
            nc.vector.tensor_copy(Os1[:], out1[:])

            # j-downsample: outf[jj, (i,v)] = sum_j wj[j,jj] cube2[96i+j, v]
            outf = psum.tile([OUT_J, ROWS_PER_CORE * NV], F32)
            for i in range(ROWS_PER_CORE):
                for m in range(4):
                    nc.tensor.matmul(outf[:, NV * i:NV * (i + 1)],
                                     smt[:, (i * 4 + m) * OUT_J:
                                         (i * 4 + m + 1) * OUT_J],
                                     Os1[:, NV * m:NV * (m + 1)],
                                     start=(m == 0), stop=(m == 3))
            outf_sb = io.tile([OUT_J, ROWS_PER_CORE * NV], F32, tag="outf_sb")
            nc.vector.tensor_copy(outf_sb[:], outf[:])
            nc.sync.dma_start(out=out[:], in_=outf_sb[:])

    return nc


def _recon_matrix(vel, sig2, si):
    """Ridge-regularized reconstruction R[NC, NV]: coarse Gaussian samples
    -> fine samples, fit over all reachable centers u."""
    vel = np.asarray(vel, np.float64).reshape(-1)
    vc = np.linspace(float(vel.min()), float(vel.max()), NC)
    umax = max(200.0 * abs(si), 1e-3)
    uu = np.linspace(-umax * 1.02, umax * 1.02, 4001)
    Ac = np.exp(-((vc[None, :] - uu[:, None]) ** 2) / sig2)
    Af = np.exp(-((vel[None, :] - uu[:, None]) ** 2) / sig2)
    R = np.linalg.solve(Ac.T @ Ac + RIDGE_LAM * np.eye(NC), Ac.T @ Af)
    return R.astype(np.float32)


def kernel(**inputs):
    inc = float(np.asarray(inputs["inclination"]).reshape(-1)[0])
    rot = float(np.asarray(inputs["sky_rot"]).reshape(-1)[0])
    lb = float(np.asarray(inputs["line_broadening"]).reshape(-1)[0])
    vel = np.asarray(inputs["velocity_grid"], np.float32).reshape(-1)
    X = np.asarray(inputs["Xgrid"], np.float32)
    Y = np.asarray(inputs["Ygrid"], np.float32)
    Z = np.asarray(inputs["Zgrid"], np.float32)

    ci, si = math.cos(inc), math.sin(inc)
    cr, sr = math.cos(rot), math.sin(rot)
    sig2 = float(np.float32(lb) * np.float32(lb))
    if not (sig2 > 0.0) or not math.isfinite(sig2):
        sig2 = 1e-30  # degenerate sigma: reference output is ~0/NaN anyway
    lnnorm = float(-0.5 * math.log(2.0 * math.pi * sig2))

    nc = _build_program(ci, si, cr, sr, sig2, lnnorm, vel)
    nc.finalize()

    vc = np.linspace(float(vel.min()), float(vel.max()), NC)
    bcv = np.ascontiguousarray(
        np.tile((-(vc.astype(np.float64) ** 2) / sig2).astype(np.float32),
                (NP128, 1)))

    # selector stationaries S_c
    scv = np.zeros((NP128, 12), np.float32)
    for c in range(3):
        for k in range(NP128):
            m = (128 * c + k) // 96
            if 0 <= m < 4 and 96 * m <= 128 * c + k < 96 * (m + 1):
                scv[k, 4 * c + m] = 1.0

    # reconstruction moving matrix W[(b,m), (m',v)] = delta R[b, v]
    R = _recon_matrix(vel, sig2, si)
    wrv = np.zeros((NP128, 4 * NV), np.float32)
    for b in range(NC):
        for m in range(4):
            wrv[4 * b + m, NV * m:NV * (m + 1)] = R[b]

    # j-downsample stencil and its zero-padded stationaries
    wj = np.zeros((G, OUT_J), np.float32)
    for m in range(OUT_J // 2):
        wj[3 * m, 2 * m] = 0.75
        wj[3 * m + 1, 2 * m] = 0.25
        wj[3 * m + 1, 2 * m + 1] = 0.25
        wj[3 * m + 2, 2 * m + 1] = 0.75
    smv = np.zeros((NT, 16 * OUT_J), np.float32)
    for i in range(4):
        for m in range(4):
            col = (i * 4 + m) * OUT_J
            for s in range(24):
                smv[24 * i + s, col:col + OUT_J] = wj[4 * s + m]

    as_bf16 = (lambda a: np.ascontiguousarray(a.astype(_BF16))) if _BF16 \
        else (lambda a: np.ascontiguousarray(a))

    in_maps = []
    for c in range(N_CORES):
        rows = [3 * k + 1 for k in range(ROWS_PER_CORE * c,
                                         ROWS_PER_CORE * (c + 1))]
        def shard(a):
            s = a[rows]                      # (4, 96, 96) = (i, j, z)
            flat = s.reshape(-1)             # flat = px*96 + z
            t = flat.reshape(NF, NP128).T    # [partition, free]
            return np.ascontiguousarray(t)
        in_maps.append({"xs": shard(X), "ys": shard(Y), "zs": shard(Z),
                        "bc": bcv, "sc": as_bf16(scv), "wr": as_bf16(wrv),
                        "sm": as_bf16(smv), "idm": as_bf16(np.eye(NT, dtype=np.float32))})

    res = run_bass_kernel_spmd(nc, in_maps, core_ids=list(range(N_CORES)))
    global LAST_EXEC_NS
    LAST_EXEC_NS = res.exec_time_ns

    parts = []
    for c in range(N_CORES):
        o = res.results[c]["out"]            # (64, 256) = [jj, i*64+v]
        parts.append(o.reshape(OUT_J, ROWS_PER_CORE, NV).transpose(1, 0, 2))
    return np.concatenate(parts, axis=0).astype(np.float32)  # (32, 64, 64)



# revision 41
# speedup vs baseline: 1.4678x; 1.4678x over previous
"""Trainium2 Bass kernel for the CubeSimulator problem (v3).

Reference: rotate (96,96,96) grids, per-voxel line-of-sight velocity u and
intensity I, Gaussian-KDE cube over 64 velocity bins, then trilinear
downsample (96,96,64) -> (32,64,64).

Structure kept from v2 (validated):
 - axis0 downsample (96->32) is a pure selection of rows 3k+1;
 - axis2 downsample (64->64) is the identity;
 - axis1 downsample (96->64) is a 2-tap stencil (0.75/0.25) matmul;
 - wrap layout [128, 288]: per-core voxels flat=(px*96+z) laid out
   partition=flat%128, free=flat//128; z-sums via selector matmuls on PE.

New in v3:
 - NC=16 coarse bins (was 28): recon centers span +-(umax+0.4*sigma)
   instead of the full velocity range, which keeps the ridge recon error
   ~7e-3 vs the 2e-2 budget (validated in numpy against the reference).
 - Merged exps: affine bins' exp arguments are precomputed in SBUF and
   processed 4 bins per Activation instruction ([128,1152]), amortizing
   the 185ns fixed ACT overhead (425ns/bin -> 286ns/bin).
 - Arg chains: arg_{b+3} = (arg_b + dcv) + Delta3 as a single fused
   scalar_tensor_tensor per bin (360ns), split across DVE and Pool.
 - Prep uses q = rx^2 + ry^2 (rotation rows), fused STT ops, and the
   eps-free Ln ladder; tanh via (e^r-1)/(r(e^r+1)) with den = (er+1)*r
   in one STT. Bins 0-3 stay in the old exp(scale*u0+bias)*P0 form so
   ACT starts right after u0 instead of waiting for the full At chain.
 - Incremental tail: cols 0-31 of the z-sum PSUM are transposed and fed
   to the recon matmul while later bins still compute.

Sharding: 32 needed i-rows split 4-per-core across 8 cores; only the
final (64, 4*64) tile is gathered per core.
"""

import math

import numpy as np

import concourse.bacc as bacc
import concourse.bass as bass
import concourse.mybir as mybir
import concourse.tile as tile
from concourse.bass_utils import run_bass_kernel_spmd

try:
    import ml_dtypes
    _BF16 = np.dtype(ml_dtypes.bfloat16)
except Exception:  # pragma: no cover
    _BF16 = None

G = 96            # up_gal grid size
NV = 64           # reference velocity bins
NC = 15           # coarse KDE bins (reconstructed to NV by matmul)
N_OLD = 7         # bins computed as exp(scale*u0+bias)*P0 (start early)
STRIDE = 4        # affine-chain stride (NC - N_OLD = 8 = 4 chains x 2)
N_CORES = 8
OUT_I = 32        # selected i rows (axis-0 downsample = row selection)
ROWS_PER_CORE = OUT_I // N_CORES   # 4
PX = ROWS_PER_CORE * G             # 384 pixels per core
NZ = G                             # z depth
NFLAT = PX * NZ                    # 36864 voxels per core
NP128 = 128
NF = NFLAT // NP128                # 288 free columns
NT = NF // 3                       # 96 column triplets (4 pixels each)
OUT_J = 64
RIDGE_LAM = 1e-5
VC_EXT = 0.3      # coarse-center span: +-(umax + VC_EXT*sigma)

F32 = mybir.dt.float32
BF16 = mybir.dt.bfloat16
AF = mybir.ActivationFunctionType
OP = mybir.AluOpType

LAST_EXEC_NS = None


def _vc_centers(si, sigma):
    umax = max(200.0 * abs(si), 1e-3)
    span = umax + VC_EXT * sigma
    return np.linspace(-span, span, NC)


def _build_program(ci, si, cr, sr, sig2, lnnorm, vel):
    sigma = math.sqrt(sig2)
    vc = _vc_centers(si, sigma)
    dv = float(vc[1] - vc[0])
    cv = -(vc.astype(np.float64) ** 2) / sig2       # per-bin scalar offsets
    usc = -200.0 * si
    esc = usc * 2.0 / sig2
    kap2 = (usc * usc) / sig2

    # rotation rows (R = Rx(inc) @ Rz(rot))
    d_, e_ = cr, -sr                    # rx row
    f_, g_, h_ = ci * sr, ci * cr, -si  # ry row
    a_, b_, c_ = si * sr, si * cr, ci   # rz row

    nc = bacc.Bacc("TRN2")

    xs = nc.dram_tensor("xs", [NP128, NF], F32, kind="ExternalInput")
    ys = nc.dram_tensor("ys", [NP128, NF], F32, kind="ExternalInput")
    zs = nc.dram_tensor("zs", [NP128, NF], F32, kind="ExternalInput")
    # bc columns: [0..N_OLD) = cv_b biases for old-style bins; [N_OLD] = -1.0
    bc = nc.dram_tensor("bc", [NP128, N_OLD + 1], F32, kind="ExternalInput")
    sc = nc.dram_tensor("sc", [NP128, 12], BF16, kind="ExternalInput")
    idm = nc.dram_tensor("idm", [NT, NT], BF16, kind="ExternalInput")
    wr = nc.dram_tensor("wr", [OUT_J, 2 * NV], BF16, kind="ExternalInput")
    sm = nc.dram_tensor("sm", [NT, 16 * OUT_J], BF16, kind="ExternalInput")
    out = nc.dram_tensor("out", [OUT_J, ROWS_PER_CORE * NV], F32,
                         kind="ExternalOutput")

    with tile.TileContext(nc) as tc:
        with (
            tc.tile_pool(name="io", bufs=1) as io,
            tc.tile_pool(name="prep", bufs=1) as prep,
            tc.tile_pool(name="kde", bufs=2) as kde,
            tc.tile_pool(name="psum", bufs=1, space="PSUM") as psum,
        ):
            # Preload the ln/exp/square/identity table once, first on the
            # ACT queue (keeps the auto-inserter from adding table swaps).
            from concourse.hw_specs import get_activation_tables
            tabs = get_activation_tables(nc.m.arch)
            want = {AF.Ln, AF.Exp, AF.Abs}
            for idx, (tname, funcs) in enumerate(tabs.items()):
                if want.issubset(funcs):
                    ld = mybir.InstLoadActFuncSet(
                        name=nc.scalar.bass.get_next_instruction_name(),
                        act_func_set_id=idx, ins=[], outs=[])
                    nc.scalar.add_instruction(ld)
                    break

            xt = io.tile([NP128, NF], F32, tag="xt")
            yt = io.tile([NP128, NF], F32, tag="yt")
            zt = io.tile([NP128, NF], F32, tag="zt")
            # Parallel DMA issue: z on SP, x via Pool SWDGE (25ns issue),
            # y on SP's second slot -- all three land by ~3.9us.
            nc.sync.dma_start(out=zt[:], in_=zs[:])
            nc.gpsimd.dma_start(out=xt[:], in_=xs[:])
            nc.sync.dma_start(out=yt[:], in_=ys[:])
            bct = io.tile([NP128, N_OLD + 1], F32, tag="bct")
            nc.sync.dma_start(out=bct[:], in_=bc[:])
            sct = io.tile([NP128, 12], BF16, tag="sct")
            nc.sync.dma_start(out=sct[:], in_=sc[:])
            idt = io.tile([NT, NT], BF16, tag="idt")
            nc.sync.dma_start(out=idt[:], in_=idm[:])
            wrt = io.tile([OUT_J, 2 * NV], BF16, tag="wrt")
            nc.sync.dma_start(out=wrt[:], in_=wr[:])
            smt = io.tile([NT, 16 * OUT_J], BF16, tag="smt")
            nc.sync.dma_start(out=smt[:], in_=sm[:])

            def vtile(name):
                return prep.tile([NP128, NF], F32, tag=name, name=name)

            V, P, S = nc.vector, nc.gpsimd, nc.scalar

            # ---- prep: u0 = rx*tanh(r/2)/r, At = L + lnnorm - kap2*u0^2 ----
            t_ry, ry1, rx1, rx, ry = (vtile(n) for n in
                                      ("t_ry", "ry1", "rx1", "rx", "ry"))
            rxq, ryq, q = vtile("rxq"), vtile("ryq"), vtile("q")
            t_rz, rza, rzb, rzc, rz = (vtile(n) for n in
                                       ("t_rz", "rza", "rzb", "rzc", "rz"))
            lnq, r = vtile("lnq"), vtile("r")
            h1, g1, h2, g2, dn, rec, t1, u0 = (vtile(n) for n in
                ("h1", "g1", "h2", "g2", "dn", "rec", "t1", "u0"))
            az, rterm, azm, Lt, ssq, At = (vtile(n) for n in
                                           ("az", "rterm", "azm", "Lt",
                                            "ssq", "At"))
            d4 = vtile("d4")

            # DVE carries the critical chain to q (z, x, y arrival order);
            # rx^2 is offloaded to ACT (Square) so q = rxq + ryq needs only
            # one DVE square.
            V.tensor_scalar_mul(t_ry[:], zt[:], h_)
            V.tensor_scalar_mul(rx1[:], xt[:], d_)
            V.scalar_tensor_tensor(ry1[:], xt[:], f_, t_ry[:], OP.mult, OP.add)
            V.scalar_tensor_tensor(rx[:], yt[:], e_, rx1[:], OP.mult, OP.add)
            S.activation(rxq[:], rx[:], AF.Square)   # rx^2 offloaded to ACT
            V.scalar_tensor_tensor(ry[:], yt[:], g_, ry1[:], OP.mult, OP.add)
            V.tensor_mul(ryq[:], ry[:], ry[:])
            V.tensor_add(q[:], rxq[:], ryq[:])
            # Pool: rz chain (off critical path; Pool has no fused STT)
            P.tensor_scalar_mul(t_rz[:], zt[:], c_)
            P.tensor_scalar_mul(rza[:], xt[:], a_)
            P.tensor_add(rzb[:], rza[:], t_rz[:])
            P.tensor_scalar_mul(rzc[:], yt[:], b_)
            P.tensor_add(rz[:], rzb[:], rzc[:])

            # r = sqrt(q) via exp(0.5*ln(q)) -- only needed for the
            # intensity term -r/3, off the u0 critical path.
            # (q = rx^2+ry^2 >= ~3e-5 on this grid -- no clamp needed)
            S.activation(lnq[:], q[:], AF.Ln)
            S.activation(r[:], lnq[:], AF.Exp, scale=0.5)

            # u0 = rx*tanh(sqrt(q)/2)/sqrt(q) via a (3,3) rational in q
            # (max rel err 6.5e-5 on q in [0,310]); Horner steps are single
            # AFFINE_MUL_REDUCE ops: out = (in0*s0 + s1)*in1.
            AQ = (4.99967744e-01, 1.48572609e-02, 6.09554350e-05,
                  2.08734598e-08)
            BQ = (1.13003511e-01, 1.21426105e-03, 1.72065489e-06)
            jnk = [prep.tile([NP128, 1], F32, tag=f"jnk{i}", name=f"jnk{i}")
                   for i in range(5)]
            V.affine_mul_reduce(h1[:], jnk[0][:], q[:], q[:], AQ[3], AQ[2])
            V.affine_mul_reduce(g1[:], jnk[1][:], q[:], q[:], BQ[2], BQ[1])
            V.affine_mul_reduce(h2[:], jnk[2][:], h1[:], q[:], 1.0, AQ[1])
            V.affine_mul_reduce(g2[:], jnk[3][:], g1[:], q[:], 1.0, BQ[0])
            V.tensor_scalar_add(dn[:], g2[:], 1.0)
            V.affine_mul_reduce(t1[:], jnk[4][:], h2[:], rx[:], 1.0, AQ[0])
            V.reciprocal(rec[:], dn[:])
            V.tensor_mul(u0[:], t1[:], rec[:])

            # At = (-r/3 - 2|rz| + lnnorm) - kap2*u0^2
            # az on ACT (fills the idle window after r; r/Lt are off the
            # u0 critical path), the rest of the Lt chain on Pool.
            S.activation(az[:], rz[:], AF.Abs)
            P.tensor_scalar(rterm[:], r[:], -1.0 / 3.0, lnnorm,
                            OP.mult, OP.add)
            P.tensor_scalar_mul(azm[:], az[:], -2.0)
            P.tensor_add(Lt[:], azm[:], rterm[:])
            V.scalar_tensor_tensor(ssq[:], u0[:], kap2, u0[:],
                                   OP.mult, OP.mult)
            V.tensor_sub(At[:], Lt[:], ssq[:])
            # Delta4 = u0 * (esc*4*dv), shared by all chains (on Pool)
            P.tensor_scalar_mul(d4[:], u0[:], esc * STRIDE * dv)

            # ---- KDE ----
            # PSUM accumulators
            Op = psum.tile([NT, 4 * NC], F32)      # z-sums per (triplet, bin)
            # outj[jj, i*32 + b]: j-downsampled z-sums; cols i*32+16..31 are
            # zero padding so the transposed i-blocks start at partition 32i
            outj = psum.tile([OUT_J, 128], F32)
            T2a = psum.tile([OUT_J, OUT_J], BF16)  # outj cols 0:64 transposed
            T2b = psum.tile([OUT_J, OUT_J], BF16)  # outj cols 64:128 transposed
            outc = psum.tile([OUT_J, ROWS_PER_CORE * NV], F32)
            # zero the pad columns once via an SBUF zero tile (the pads are
            # read by the transposes)
            zpad = io.tile([OUT_J, 32 - NC], F32, tag="zpad")
            V.memset(zpad[:], 0.0)
            for i in range(ROWS_PER_CORE):
                V.tensor_copy(outj[:, 32 * i + NC:32 * i + 32], zpad[:])

            def sel_matmuls(b, mov_c):
                # mov_c(c): [128, 96] moving slice for bin b, triplet lane c
                for c in range(3):
                    nc.tensor.matmul(Op[:, 4 * b:4 * b + 4],
                                     mov_c(c),
                                     sct[:, 4 * c:4 * c + 4],
                                     start=(c == 0), stop=(c == 2))

            # Old-style bins 0..N_OLD-1: P0 = exp(At), e1_b = exp(scale*u0+cv)
            # P0 is emitted after the first two e1 exps so the e1 stream
            # starts the moment u0 lands; mults wait for P0 anyway.
            P0b = kde.tile([NP128, NF], BF16, tag="P0b", name="P0b", bufs=1)
            e1s = []
            for b in range(N_OLD):
                e1 = kde.tile([NP128, NT, 3], BF16, tag="e1", bufs=N_OLD,
                              name=f"e1{b}")
                S.activation(e1[:], u0[:], AF.Exp, scale=float(vc[b]) * esc,
                             bias=bct[:, b:b + 1])
                e1s.append(e1)
                if b == 1:
                    S.activation(P0b[:], At[:], AF.Exp)

            # Affine bins N_OLD..NC-1: 4 chains of 2 (seed + one step),
            # seeds + steps all on DVE (Pool cannot run fused STT).
            GW = NC - N_OLD              # 8 affine bins
            NGRP = GW // 4               # 2 merged groups of 4
            argts = [kde.tile([NP128, 4, NF], F32, tag=f"arg{g}", bufs=1,
                              name=f"arg{g}") for g in range(NGRP)]

            def argv(b):
                g, j = (b - N_OLD) // 4, (b - N_OLD) % 4
                return argts[g][:, j, :]

            for k in range(STRIDE):
                bb = N_OLD + k
                V.affine_then_add(argv(bb), u0[:], At[:],
                                  float(vc[bb]) * esc, float(cv[bb]))
            for k in range(STRIDE):
                b = N_OLD + STRIDE + k
                V.scalar_tensor_tensor(
                    argv(b), argv(b - STRIDE),
                    float(cv[b] - cv[b - STRIDE]), d4[:], OP.add, OP.add)

            # P0 mults for old bins: first half DVE (bf16 2x), rest Pool
            Ewold = []
            for b in range(N_OLD):
                Ew = kde.tile([NP128, NT, 3], BF16, tag="Ewo", bufs=N_OLD,
                              name=f"Ewo{b}")
                eng = V if b < 4 else P
                eng.tensor_mul(Ew[:], e1s[b][:], P0b[:])
                Ewold.append(Ew)
                sel_matmuls(b, lambda c, _E=Ew: _E[:, :, c])

            def outj_matmuls(chunk, OpsM):
                # OpsM holds the chunk's Op columns repacked m-major
                # outj[jj, i*32+chunk*8+b'] += sum_t smt_im[t,jj]*OpsM[t,m*8+b']
                for i in range(ROWS_PER_CORE):
                    for m in range(4):
                        nc.tensor.matmul(
                            outj[:, i * 32 + chunk * 8:i * 32 + chunk * 8 + 8],
                            smt[:, (i * 4 + m) * OUT_J:(i * 4 + m + 1) * OUT_J],
                            OpsM[:, 8 * m:8 * m + 8],
                            start=(m == 0), stop=(m == 3))

            # merged exps + sel matmuls per group
            for g in range(NGRP):
                Ewg = kde.tile([NP128, 4, NT, 3], BF16, tag="Ewg", bufs=2,
                               name=f"Ewg{g}")
                S.activation(Ewg[:], argts[g][:], AF.Exp)
                for j in range(4):
                    sel_matmuls(N_OLD + 4 * g + j,
                                lambda c, _E=Ewg, _j=j: _E[:, _j, :, c])
                if g == 0:
                    # chunk A: bins 0..7 (cols 0..31; olds + affine bin 7)
                    OpsAm = io.tile([NT, 32], BF16, tag="OpsAm")
                    V.tensor_copy(
                        OpsAm[:].rearrange("p (m b) -> p m b", m=4),
                        Op[:, 0:32].rearrange("p (b m) -> p m b", m=4))
                    outj_matmuls(0, OpsAm)

            # chunk B: remaining affine bins (cols 32..4*NC)
            NB2 = NC - 8
            OpsBm = io.tile([NT, 4 * NB2], BF16, tag="OpsBm")
            V.tensor_copy(OpsBm[:].rearrange("p (m b) -> p m b", m=4),
                          Op[:, 32:4 * NC].rearrange("p (b m) -> p m b", m=4))
            for i in range(ROWS_PER_CORE):
                for m in range(4):
                    nc.tensor.matmul(
                        outj[:, i * 32 + 8:i * 32 + 8 + NB2],
                        smt[:, (i * 4 + m) * OUT_J:(i * 4 + m + 1) * OUT_J],
                        OpsBm[:, NB2 * m:NB2 * m + NB2],
                        start=(m == 0), stop=(m == 3))

            # transpose outj in two base-0 halves (base-64 matmul operands
            # are rejected by the device), then recon per i-pair:
            # cube[jj, (i,v)] = sum_b T[32(i%2)+b, jj] * R[b, v]
            outjs = io.tile([OUT_J, 128], BF16, tag="outjs")
            V.tensor_copy(outjs[:], outj[:])
            id64 = idt[0:OUT_J, 0:OUT_J]
            nc.tensor.transpose(T2a[:, :], outjs[:, 0:OUT_J], id64)
            nc.tensor.transpose(T2b[:, :], outjs[:, OUT_J:128], id64)
            t2a = io.tile([OUT_J, OUT_J], BF16, tag="t2a")
            t2b = io.tile([OUT_J, OUT_J], BF16, tag="t2b")
            V.tensor_copy(t2a[:], T2a[:])
            V.tensor_copy(t2b[:], T2b[:])
            nc.tensor.matmul(outc[:, 0:2 * NV], t2a[:], wrt[:],
                             start=True, stop=True)
            nc.tensor.matmul(outc[:, 2 * NV:4 * NV], t2b[:], wrt[:],
                             start=True, stop=True)
            outf_sb = io.tile([OUT_J, ROWS_PER_CORE * NV], F32, tag="outf_sb")
            S.activation(outf_sb[:], outc[:], AF.Copy)
            nc.sync.dma_start(out=out[:], in_=outf_sb[:])

    return nc


def _recon_matrix(vel, sig2, si):
    """Ridge-regularized reconstruction R[NC, NV]: coarse Gaussian samples
    -> fine samples, fit over all reachable centers u."""
    vel = np.asarray(vel, np.float64).reshape(-1)
    vc = _vc_centers(si, math.sqrt(sig2))
    umax = max(200.0 * abs(si), 1e-3)
    uu = np.linspace(-umax * 1.02, umax * 1.02, 4001)
    Ac = np.exp(-((vc[None, :] - uu[:, None]) ** 2) / sig2)
    Af = np.exp(-((vel[None, :] - uu[:, None]) ** 2) / sig2)
    R = np.linalg.solve(Ac.T @ Ac + RIDGE_LAM * np.eye(NC), Ac.T @ Af)
    return R.astype(np.float32)


def kernel(**inputs):
    inc = float(np.asarray(inputs["inclination"]).reshape(-1)[0])
    rot = float(np.asarray(inputs["sky_rot"]).reshape(-1)[0])
    lb = float(np.asarray(inputs["line_broadening"]).reshape(-1)[0])
    vel = np.asarray(inputs["velocity_grid"], np.float32).reshape(-1)
    X = np.asarray(inputs["Xgrid"], np.float32)
    Y = np.asarray(inputs["Ygrid"], np.float32)
    Z = np.asarray(inputs["Zgrid"], np.float32)

    ci, si = math.cos(inc), math.sin(inc)
    cr, sr = math.cos(rot), math.sin(rot)
    sig2 = float(np.float32(lb) * np.float32(lb))
    if not (sig2 > 0.0) or not math.isfinite(sig2):
        sig2 = 1e-30  # degenerate sigma: reference output is ~0/NaN anyway
    lnnorm = float(-0.5 * math.log(2.0 * math.pi * sig2))

    nc = _build_program(ci, si, cr, sr, sig2, lnnorm, vel)
    nc.finalize()

    vc = _vc_centers(si, math.sqrt(sig2))
    bcols = np.concatenate([
        (-(vc[:N_OLD].astype(np.float64) ** 2) / sig2).astype(np.float32),
        np.float32([-1.0])])
    bcv = np.ascontiguousarray(np.tile(bcols, (NP128, 1)))

    # selector stationaries S_c
    scv = np.zeros((NP128, 12), np.float32)
    for c in range(3):
        for k in range(NP128):
            m = (128 * c + k) // 96
            if 0 <= m < 4 and 96 * m <= 128 * c + k < 96 * (m + 1):
                scv[k, 4 * c + m] = 1.0

    # reconstruction stationary, block-diagonal over the i-pair: rows
    # 32d+b map to column block d*NV with values R[b]; pad rows stay zero
    R = _recon_matrix(vel, sig2, si)
    wrv = np.zeros((OUT_J, 2 * NV), np.float32)
    for dblk in range(2):
        wrv[32 * dblk:32 * dblk + NC, dblk * NV:(dblk + 1) * NV] = R

    # j-downsample stencil and its zero-padded stationaries
    wj = np.zeros((G, OUT_J), np.float32)
    for m in range(OUT_J // 2):
        wj[3 * m, 2 * m] = 0.75
        wj[3 * m + 1, 2 * m] = 0.25
        wj[3 * m + 1, 2 * m + 1] = 0.25
        wj[3 * m + 2, 2 * m + 1] = 0.75
    smv = np.zeros((NT, 16 * OUT_J), np.float32)
    for i in range(4):
        for m in range(4):
            col = (i * 4 + m) * OUT_J
            for s in range(24):
                smv[24 * i + s, col:col + OUT_J] = wj[4 * s + m]

    as_bf16 = (lambda a: np.ascontiguousarray(a.astype(_BF16))) if _BF16 \
        else (lambda a: np.ascontiguousarray(a))

    in_maps = []
    for c in range(N_CORES):
        rows = [3 * k + 1 for k in range(ROWS_PER_CORE * c,
                                         ROWS_PER_CORE * (c + 1))]
        def shard(a):
            s = a[rows]                      # (4, 96, 96) = (i, j, z)
            flat = s.reshape(-1)             # flat = px*96 + z
            t = flat.reshape(NF, NP128).T    # [partition, free]
            return np.ascontiguousarray(t)
        in_maps.append({"xs": shard(X), "ys": shard(Y), "zs": shard(Z),
                        "bc": bcv, "sc": as_bf16(scv), "wr": as_bf16(wrv),
                        "sm": as_bf16(smv),
                        "idm": as_bf16(np.eye(NT, dtype=np.float32))})

    res = run_bass_kernel_spmd(nc, in_maps, core_ids=list(range(N_CORES)))
    global LAST_EXEC_NS
    LAST_EXEC_NS = res.exec_time_ns

    parts = []
    for c in range(N_CORES):
        o = res.results[c]["out"]            # (64, 256) = [jj, i*64+v]
        parts.append(o.reshape(OUT_J, ROWS_PER_CORE, NV).transpose(1, 0, 2))
    return np.concatenate(parts, axis=0).astype(np.float32)  # (32, 64, 64)


# revision 45
# speedup vs baseline: 1.4924x; 1.0168x over previous
"""Trainium2 Bass kernel for the CubeSimulator problem (v3).

Reference: rotate (96,96,96) grids, per-voxel line-of-sight velocity u and
intensity I, Gaussian-KDE cube over 64 velocity bins, then trilinear
downsample (96,96,64) -> (32,64,64).

Structure kept from v2 (validated):
 - axis0 downsample (96->32) is a pure selection of rows 3k+1;
 - axis2 downsample (64->64) is the identity;
 - axis1 downsample (96->64) is a 2-tap stencil (0.75/0.25) matmul;
 - wrap layout [128, 288]: per-core voxels flat=(px*96+z) laid out
   partition=flat%128, free=flat//128; z-sums via selector matmuls on PE.

New in v3:
 - NC=16 coarse bins (was 28): recon centers span +-(umax+0.4*sigma)
   instead of the full velocity range, which keeps the ridge recon error
   ~7e-3 vs the 2e-2 budget (validated in numpy against the reference).
 - Merged exps: affine bins' exp arguments are precomputed in SBUF and
   processed 4 bins per Activation instruction ([128,1152]), amortizing
   the 185ns fixed ACT overhead (425ns/bin -> 286ns/bin).
 - Arg chains: arg_{b+3} = (arg_b + dcv) + Delta3 as a single fused
   scalar_tensor_tensor per bin (360ns), split across DVE and Pool.
 - Prep uses q = rx^2 + ry^2 (rotation rows), fused STT ops, and the
   eps-free Ln ladder; tanh via (e^r-1)/(r(e^r+1)) with den = (er+1)*r
   in one STT. Bins 0-3 stay in the old exp(scale*u0+bias)*P0 form so
   ACT starts right after u0 instead of waiting for the full At chain.
 - Incremental tail: cols 0-31 of the z-sum PSUM are transposed and fed
   to the recon matmul while later bins still compute.

Sharding: 32 needed i-rows split 4-per-core across 8 cores; only the
final (64, 4*64) tile is gathered per core.
"""

import math

import numpy as np

import concourse.bacc as bacc
import concourse.bass as bass
import concourse.mybir as mybir
import concourse.tile as tile
from concourse.bass_utils import run_bass_kernel_spmd

try:
    import ml_dtypes
    _BF16 = np.dtype(ml_dtypes.bfloat16)
except Exception:  # pragma: no cover
    _BF16 = None

G = 96            # up_gal grid size
NV = 64           # reference velocity bins
NC = 15           # coarse KDE bins (reconstructed to NV by matmul)
N_OLD = 7         # bins computed as exp(scale*u0+bias)*P0 (start early)
STRIDE = 4        # affine-chain stride (NC - N_OLD = 8 = 4 chains x 2)
N_CORES = 8
OUT_I = 32        # selected i rows (axis-0 downsample = row selection)
ROWS_PER_CORE = OUT_I // N_CORES   # 4
PX = ROWS_PER_CORE * G             # 384 pixels per core
NZ = G                             # z depth
NFLAT = PX * NZ                    # 36864 voxels per core
NP128 = 128
NF = NFLAT // NP128                # 288 free columns
NT = NF // 3                       # 96 column triplets (4 pixels each)
OUT_J = 64
RIDGE_LAM = 1e-5
VC_EXT = 0.3      # coarse-center span: +-(umax + VC_EXT*sigma)

F32 = mybir.dt.float32
BF16 = mybir.dt.bfloat16
AF = mybir.ActivationFunctionType
OP = mybir.AluOpType

LAST_EXEC_NS = None


def _vc_centers(si, sigma):
    umax = max(200.0 * abs(si), 1e-3)
    span = umax + VC_EXT * sigma
    return np.linspace(-span, span, NC)


def _build_program(ci, si, cr, sr, sig2, lnnorm, vel):
    sigma = math.sqrt(sig2)
    vc = _vc_centers(si, sigma)
    dv = float(vc[1] - vc[0])
    cv = -(vc.astype(np.float64) ** 2) / sig2       # per-bin scalar offsets
    usc = -200.0 * si
    esc = usc * 2.0 / sig2
    kap2 = (usc * usc) / sig2

    # rotation rows (R = Rx(inc) @ Rz(rot))
    d_, e_ = cr, -sr                    # rx row
    f_, g_, h_ = ci * sr, ci * cr, -si  # ry row
    a_, b_, c_ = si * sr, si * cr, ci   # rz row

    nc = bacc.Bacc("TRN2")

    xs = nc.dram_tensor("xs", [NP128, NF], F32, kind="ExternalInput")
    ys = nc.dram_tensor("ys", [NP128, NF], F32, kind="ExternalInput")
    zs = nc.dram_tensor("zs", [NP128, NF], F32, kind="ExternalInput")
    # bc columns: [0..N_OLD) = cv_b biases for old-style bins; [N_OLD] = -1.0
    bc = nc.dram_tensor("bc", [NP128, N_OLD + 1], F32, kind="ExternalInput")
    sc = nc.dram_tensor("sc", [NP128, 12], BF16, kind="ExternalInput")
    idm = nc.dram_tensor("idm", [NT, NT], BF16, kind="ExternalInput")
    wr = nc.dram_tensor("wr", [OUT_J, 2 * NV], BF16, kind="ExternalInput")
    sm = nc.dram_tensor("sm", [NT, 16 * OUT_J], BF16, kind="ExternalInput")
    out = nc.dram_tensor("out", [OUT_J, ROWS_PER_CORE * NV], F32,
                         kind="ExternalOutput")

    with tile.TileContext(nc) as tc:
        with (
            tc.tile_pool(name="io", bufs=1) as io,
            tc.tile_pool(name="prep", bufs=1) as prep,
            tc.tile_pool(name="kde", bufs=2) as kde,
            tc.tile_pool(name="psum", bufs=1, space="PSUM") as psum,
        ):
            # Preload the ln/exp/square/identity table once, first on the
            # ACT queue (keeps the auto-inserter from adding table swaps).
            from concourse.hw_specs import get_activation_tables
            tabs = get_activation_tables(nc.m.arch)
            want = {AF.Ln, AF.Exp, AF.Abs}
            for idx, (tname, funcs) in enumerate(tabs.items()):
                if want.issubset(funcs):
                    ld = mybir.InstLoadActFuncSet(
                        name=nc.scalar.bass.get_next_instruction_name(),
                        act_func_set_id=idx, ins=[], outs=[])
                    nc.scalar.add_instruction(ld)
                    break

            xt = io.tile([NP128, NF], F32, tag="xt")
            yt = io.tile([NP128, NF], F32, tag="yt")
            zt = io.tile([NP128, NF], F32, tag="zt")
            # Parallel DMA issue: z on SP, x via Pool SWDGE (25ns issue),
            # y on SP's second slot -- all three land by ~3.9us.
            nc.sync.dma_start(out=zt[:], in_=zs[:])
            nc.gpsimd.dma_start(out=xt[:], in_=xs[:])
            nc.sync.dma_start(out=yt[:], in_=ys[:])
            bct = io.tile([NP128, N_OLD + 1], F32, tag="bct")
            nc.sync.dma_start(out=bct[:], in_=bc[:])
            sct = io.tile([NP128, 12], BF16, tag="sct")
            nc.sync.dma_start(out=sct[:], in_=sc[:])
            idt = io.tile([NT, NT], BF16, tag="idt")
            nc.sync.dma_start(out=idt[:], in_=idm[:])
            wrt = io.tile([OUT_J, 2 * NV], BF16, tag="wrt")
            nc.sync.dma_start(out=wrt[:], in_=wr[:])
            smt = io.tile([NT, 16 * OUT_J], BF16, tag="smt")
            nc.sync.dma_start(out=smt[:], in_=sm[:])

            def vtile(name):
                return prep.tile([NP128, NF], F32, tag=name, name=name)

            V, P, S = nc.vector, nc.gpsimd, nc.scalar

            # ---- prep: u0 = rx*tanh(r/2)/r, At = L + lnnorm - kap2*u0^2 ----
            t_ry, ry1, rx1, rx, ry = (vtile(n) for n in
                                      ("t_ry", "ry1", "rx1", "rx", "ry"))
            rxq, ryq, q = vtile("rxq"), vtile("ryq"), vtile("q")
            t_rz, rza, rzb, rzc, rz = (vtile(n) for n in
                                       ("t_rz", "rza", "rzb", "rzc", "rz"))
            lnq, r = vtile("lnq"), vtile("r")
            h1, g1, h2, g2, dn, rec, t1, u0 = (vtile(n) for n in
                ("h1", "g1", "h2", "g2", "dn", "rec", "t1", "u0"))
            az, rterm, azm, Lt, ssq, At = (vtile(n) for n in
                                           ("az", "rterm", "azm", "Lt",
                                            "ssq", "At"))
            d4 = vtile("d4")

            # DVE carries the critical chain to q (z, x, y arrival order);
            # rx^2 is offloaded to ACT (Square) so q = rxq + ryq needs only
            # one DVE square.
            V.tensor_scalar_mul(t_ry[:], zt[:], h_)
            V.tensor_scalar_mul(rx1[:], xt[:], d_)
            V.scalar_tensor_tensor(ry1[:], xt[:], f_, t_ry[:], OP.mult, OP.add)
            V.scalar_tensor_tensor(rx[:], yt[:], e_, rx1[:], OP.mult, OP.add)
            S.activation(rxq[:], rx[:], AF.Square)   # rx^2 offloaded to ACT
            V.scalar_tensor_tensor(ry[:], yt[:], g_, ry1[:], OP.mult, OP.add)
            V.tensor_mul(ryq[:], ry[:], ry[:])
            V.tensor_add(q[:], rxq[:], ryq[:])
            # Pool: rz chain (off critical path; Pool has no fused STT)
            P.tensor_scalar_mul(t_rz[:], zt[:], c_)
            P.tensor_scalar_mul(rza[:], xt[:], a_)
            P.tensor_add(rzb[:], rza[:], t_rz[:])
            P.tensor_scalar_mul(rzc[:], yt[:], b_)
            P.tensor_add(rz[:], rzb[:], rzc[:])

            # r = sqrt(q) via exp(0.5*ln(q)) -- only needed for the
            # intensity term -r/3, off the u0 critical path.
            # (q = rx^2+ry^2 >= ~3e-5 on this grid -- no clamp needed)
            S.activation(lnq[:], q[:], AF.Ln)
            S.activation(r[:], lnq[:], AF.Exp, scale=0.5)

            # u0 = rx*tanh(sqrt(q)/2)/sqrt(q) via a (2,3) rational in q
            # (max rel err 7.2e-4 on q in [0,310], negligible vs the recon
            # bias); Horner steps are single AFFINE_MUL_REDUCE ops:
            # out = (in0*s0 + s1)*in1.
            AQ = (4.99639571e-01, 1.31065944e-02, 3.25724847e-05)
            BQ = (1.09142981e-01, 8.95969004e-04, 4.87126214e-07)
            jnk = [prep.tile([NP128, 1], F32, tag=f"jnk{i}", name=f"jnk{i}")
                   for i in range(4)]
            V.affine_mul_reduce(h2[:], jnk[0][:], q[:], q[:], AQ[2], AQ[1])
            V.affine_mul_reduce(g1[:], jnk[1][:], q[:], q[:], BQ[2], BQ[1])
            V.affine_mul_reduce(g2[:], jnk[2][:], g1[:], q[:], 1.0, BQ[0])
            V.tensor_scalar_add(dn[:], g2[:], 1.0)
            V.affine_mul_reduce(t1[:], jnk[3][:], h2[:], rx[:], 1.0, AQ[0])
            V.reciprocal(rec[:], dn[:])
            V.tensor_mul(u0[:], t1[:], rec[:])

            # At = (-r/3 - 2|rz| + lnnorm) - kap2*u0^2
            # az on ACT (fills the idle window after r; r/Lt are off the
            # u0 critical path), the rest of the Lt chain on Pool.
            S.activation(az[:], rz[:], AF.Abs)
            P.tensor_scalar(rterm[:], r[:], -1.0 / 3.0, lnnorm,
                            OP.mult, OP.add)
            P.tensor_scalar_mul(azm[:], az[:], -2.0)
            P.tensor_add(Lt[:], azm[:], rterm[:])
            V.scalar_tensor_tensor(ssq[:], u0[:], kap2, u0[:],
                                   OP.mult, OP.mult)
            V.tensor_sub(At[:], Lt[:], ssq[:])
            # Delta4 = u0 * (esc*4*dv), shared by all chains (on Pool)
            P.tensor_scalar_mul(d4[:], u0[:], esc * STRIDE * dv)

            # ---- KDE ----
            # PSUM accumulators
            Op = psum.tile([NT, 4 * NC], F32)      # z-sums per (triplet, bin)
            # outj[jj, i*32 + b]: j-downsampled z-sums; cols i*32+16..31 are
            # zero padding so the transposed i-blocks start at partition 32i
            outj = psum.tile([OUT_J, 128], F32)
            T2ab = psum.tile([OUT_J, 128], BF16)   # both outj halves, transposed
            outc = psum.tile([OUT_J, ROWS_PER_CORE * NV], F32)
            # zero the pad columns once via an SBUF zero tile (the pads are
            # read by the transposes)
            zpad = io.tile([OUT_J, 32 - NC], F32, tag="zpad")
            V.memset(zpad[:], 0.0)
            for i in range(ROWS_PER_CORE):
                V.tensor_copy(outj[:, 32 * i + NC:32 * i + 32], zpad[:])

            def sel_matmuls(b, mov_c):
                # mov_c(c): [128, 96] moving slice for bin b, triplet lane c
                for c in range(3):
                    nc.tensor.matmul(Op[:, 4 * b:4 * b + 4],
                                     mov_c(c),
                                     sct[:, 4 * c:4 * c + 4],
                                     start=(c == 0), stop=(c == 2))

            # Old-style bins 0..N_OLD-1: P0 = exp(At), e1_b = exp(scale*u0+cv)
            # P0 is emitted after the first two e1 exps so the e1 stream
            # starts the moment u0 lands; mults wait for P0 anyway.
            P0b = kde.tile([NP128, NF], BF16, tag="P0b", name="P0b", bufs=1)
            e1s = []
            for b in range(N_OLD):
                e1 = kde.tile([NP128, NT, 3], BF16, tag="e1", bufs=N_OLD,
                              name=f"e1{b}")
                S.activation(e1[:], u0[:], AF.Exp, scale=float(vc[b]) * esc,
                             bias=bct[:, b:b + 1])
                e1s.append(e1)
                if b == 1:
                    S.activation(P0b[:], At[:], AF.Exp)

            # Affine bins N_OLD..NC-1: 4 chains of 2 (seed + one step),
            # seeds + steps all on DVE (Pool cannot run fused STT).
            GW = NC - N_OLD              # 8 affine bins
            NGRP = GW // 4               # 2 merged groups of 4
            argts = [kde.tile([NP128, 4, NF], F32, tag=f"arg{g}", bufs=1,
                              name=f"arg{g}") for g in range(NGRP)]

            def argv(b):
                g, j = (b - N_OLD) // 4, (b - N_OLD) % 4
                return argts[g][:, j, :]

            for k in range(STRIDE):
                bb = N_OLD + k
                V.affine_then_add(argv(bb), u0[:], At[:],
                                  float(vc[bb]) * esc, float(cv[bb]))
            for k in range(STRIDE):
                b = N_OLD + STRIDE + k
                V.scalar_tensor_tensor(
                    argv(b), argv(b - STRIDE),
                    float(cv[b] - cv[b - STRIDE]), d4[:], OP.add, OP.add)

            # P0 mults for old bins: first half DVE (bf16 2x), rest Pool
            Ewold = []
            for b in range(N_OLD):
                Ew = kde.tile([NP128, NT, 3], BF16, tag="Ewo", bufs=N_OLD,
                              name=f"Ewo{b}")
                eng = V if b < 4 else P
                eng.tensor_mul(Ew[:], e1s[b][:], P0b[:])
                Ewold.append(Ew)
                sel_matmuls(b, lambda c, _E=Ew: _E[:, :, c])

            def outj_matmuls(chunk, OpsM):
                # OpsM holds the chunk's Op columns repacked m-major
                # outj[jj, i*32+chunk*8+b'] += sum_t smt_im[t,jj]*OpsM[t,m*8+b']
                for i in range(ROWS_PER_CORE):
                    for m in range(4):
                        nc.tensor.matmul(
                            outj[:, i * 32 + chunk * 8:i * 32 + chunk * 8 + 8],
                            smt[:, (i * 4 + m) * OUT_J:(i * 4 + m + 1) * OUT_J],
                            OpsM[:, 8 * m:8 * m + 8],
                            start=(m == 0), stop=(m == 3))

            # merged exps + sel matmuls per group
            for g in range(NGRP):
                Ewg = kde.tile([NP128, 4, NT, 3], BF16, tag="Ewg", bufs=2,
                               name=f"Ewg{g}")
                S.activation(Ewg[:], argts[g][:], AF.Exp)
                for j in range(4):
                    sel_matmuls(N_OLD + 4 * g + j,
                                lambda c, _E=Ewg, _j=j: _E[:, _j, :, c])
                if g == 0:
                    # chunk A: bins 0..7 (cols 0..31; olds + affine bin 7)
                    OpsAm = io.tile([NT, 32], BF16, tag="OpsAm")
                    V.tensor_copy(
                        OpsAm[:].rearrange("p (m b) -> p m b", m=4),
                        Op[:, 0:32].rearrange("p (b m) -> p m b", m=4))
                    outj_matmuls(0, OpsAm)

            # chunk B: remaining affine bins (cols 32..4*NC)
            NB2 = NC - 8
            OpsBm = io.tile([NT, 4 * NB2], BF16, tag="OpsBm")
            V.tensor_copy(OpsBm[:].rearrange("p (m b) -> p m b", m=4),
                          Op[:, 32:4 * NC].rearrange("p (b m) -> p m b", m=4))
            for i in range(ROWS_PER_CORE):
                for m in range(4):
                    nc.tensor.matmul(
                        outj[:, i * 32 + 8:i * 32 + 8 + NB2],
                        smt[:, (i * 4 + m) * OUT_J:(i * 4 + m + 1) * OUT_J],
                        OpsBm[:, NB2 * m:NB2 * m + NB2],
                        start=(m == 0), stop=(m == 3))

            # transpose outj in two base-0 halves (base-64 matmul operands
            # are rejected by the device), then recon per i-pair:
            # cube[jj, (i,v)] = sum_b T[32(i%2)+b, jj] * R[b, v]
            outjs = io.tile([OUT_J, 128], BF16, tag="outjs")
            V.tensor_copy(outjs[:], outj[:])
            id64 = idt[0:OUT_J, 0:OUT_J]
            nc.tensor.transpose(T2ab[:, 0:OUT_J], outjs[:, 0:OUT_J], id64)
            nc.tensor.transpose(T2ab[:, OUT_J:128], outjs[:, OUT_J:128], id64)
            t2ab = io.tile([OUT_J, 128], BF16, tag="t2ab")
            V.tensor_copy(t2ab[:], T2ab[:])
            nc.tensor.matmul(outc[:, 0:2 * NV], t2ab[:, 0:OUT_J], wrt[:],
                             start=True, stop=True)
            nc.tensor.matmul(outc[:, 2 * NV:4 * NV], t2ab[:, OUT_J:128],
                             wrt[:], start=True, stop=True)
            outf_sb = io.tile([OUT_J, ROWS_PER_CORE * NV], F32, tag="outf_sb")
            S.activation(outf_sb[:], outc[:], AF.Copy)
            nc.sync.dma_start(out=out[:], in_=outf_sb[:])

    return nc


def _recon_matrix(vel, sig2, si):
    """Ridge-regularized reconstruction R[NC, NV]: coarse Gaussian samples
    -> fine samples, fit over all reachable centers u."""
    vel = np.asarray(vel, np.float64).reshape(-1)
    vc = _vc_centers(si, math.sqrt(sig2))
    umax = max(200.0 * abs(si), 1e-3)
    uu = np.linspace(-umax * 1.02, umax * 1.02, 4001)
    Ac = np.exp(-((vc[None, :] - uu[:, None]) ** 2) / sig2)
    Af = np.exp(-((vel[None, :] - uu[:, None]) ** 2) / sig2)
    R = np.linalg.solve(Ac.T @ Ac + RIDGE_LAM * np.eye(NC), Ac.T @ Af)
    return R.astype(np.float32)


def kernel(**inputs):
    inc = float(np.asarray(inputs["inclination"]).reshape(-1)[0])
    rot = float(np.asarray(inputs["sky_rot"]).reshape(-1)[0])
    lb = float(np.asarray(inputs["line_broadening"]).reshape(-1)[0])
    vel = np.asarray(inputs["velocity_grid"], np.float32).reshape(-1)
    X = np.asarray(inputs["Xgrid"], np.float32)
    Y = np.asarray(inputs["Ygrid"], np.float32)
    Z = np.asarray(inputs["Zgrid"], np.float32)

    ci, si = math.cos(inc), math.sin(inc)
    cr, sr = math.cos(rot), math.sin(rot)
    sig2 = float(np.float32(lb) * np.float32(lb))
    if not (sig2 > 0.0) or not math.isfinite(sig2):
        sig2 = 1e-30  # degenerate sigma: reference output is ~0/NaN anyway
    lnnorm = float(-0.5 * math.log(2.0 * math.pi * sig2))

    nc = _build_program(ci, si, cr, sr, sig2, lnnorm, vel)
    nc.finalize()

    vc = _vc_centers(si, math.sqrt(sig2))
    bcols = np.concatenate([
        (-(vc[:N_OLD].astype(np.float64) ** 2) / sig2).astype(np.float32),
        np.float32([-1.0])])
    bcv = np.ascontiguousarray(np.tile(bcols, (NP128, 1)))

    # selector stationaries S_c
    scv = np.zeros((NP128, 12), np.float32)
    for c in range(3):
        for k in range(NP128):
            m = (128 * c + k) // 96
            if 0 <= m < 4 and 96 * m <= 128 * c + k < 96 * (m + 1):
                scv[k, 4 * c + m] = 1.0

    # reconstruction stationary, block-diagonal over the i-pair: rows
    # 32d+b map to column block d*NV with values R[b]; pad rows stay zero
    R = _recon_matrix(vel, sig2, si)
    wrv = np.zeros((OUT_J, 2 * NV), np.float32)
    for dblk in range(2):
        wrv[32 * dblk:32 * dblk + NC, dblk * NV:(dblk + 1) * NV] = R

    # j-downsample stencil and its zero-padded stationaries
    wj = np.zeros((G, OUT_J), np.float32)
    for m in range(OUT_J // 2):
        wj[3 * m, 2 * m] = 0.75
        wj[3 * m + 1, 2 * m] = 0.25
        wj[3 * m + 1, 2 * m + 1] = 0.25
        wj[3 * m + 2, 2 * m + 1] = 0.75
    smv = np.zeros((NT, 16 * OUT_J), np.float32)
    for i in range(4):
        for m in range(4):
            col = (i * 4 + m) * OUT_J
            for s in range(24):
                smv[24 * i + s, col:col + OUT_J] = wj[4 * s + m]

    as_bf16 = (lambda a: np.ascontiguousarray(a.astype(_BF16))) if _BF16 \
        else (lambda a: np.ascontiguousarray(a))

    in_maps = []
    for c in range(N_CORES):
        rows = [3 * k + 1 for k in range(ROWS_PER_CORE * c,
                                         ROWS_PER_CORE * (c + 1))]
        def shard(a):
            s = a[rows]                      # (4, 96, 96) = (i, j, z)
            flat = s.reshape(-1)             # flat = px*96 + z
            t = flat.reshape(NF, NP128).T    # [partition, free]
            return np.ascontiguousarray(t)
        in_maps.append({"xs": shard(X), "ys": shard(Y), "zs": shard(Z),
                        "bc": bcv, "sc": as_bf16(scv), "wr": as_bf16(wrv),
                        "sm": as_bf16(smv),
                        "idm": as_bf16(np.eye(NT, dtype=np.float32))})

    res = run_bass_kernel_spmd(nc, in_maps, core_ids=list(range(N_CORES)))
    global LAST_EXEC_NS
    LAST_EXEC_NS = res.exec_time_ns

    parts = []
    for c in range(N_CORES):
        o = res.results[c]["out"]            # (64, 256) = [jj, i*64+v]
        parts.append(o.reshape(OUT_J, ROWS_PER_CORE, NV).transpose(1, 0, 2))
    return np.concatenate(parts, axis=0).astype(np.float32)  # (32, 64, 64)
